# revision 1
# baseline (speedup 1.0000x reference)
"""Fused transformer block (LN -> 16-head causal attention -> proj -> residual
-> LN -> FFN -> residual) on 8 TRN2 NeuronCores.

Sharding: tokens are sharded across cores for LN/QKV/proj/FFN (512 rows of the
flattened [4096, 1024] each); attention is head-sharded (2 heads per core) so
every core runs an identical SPMD program over the full causal triangle.
AllToAll collectives redistribute Q^T/K^T/V (token-sharded -> head-sharded)
and the attention output (head-sharded -> token-sharded).

All matmuls keep operands pre-transposed so the contraction dim is always the
SBUF partition dim: h and h2 are transposed on-chip via the PE; Q/K are
produced directly in [head_dim, token] layout; attention scores are computed
transposed ([key, query]) so the softmax-weighted V accumulation is a plain
PSUM matmul chain whose appended ones-column yields the softmax denominator.
LayerNorm gains are folded into the following weight matrices host-side;
LayerNorm biases become per-output-channel biases applied on PSUM eviction
or via rank-1 ones-row matmuls.
"""

import ml_dtypes
import numpy as np

import concourse.bass as bass  # noqa: F401  (AP helpers via handles)
import concourse.mybir as mybir
import concourse.tile as tile
from concourse import bacc
from concourse.bass_utils import run_bass_kernel_spmd

F32 = mybir.dt.float32
F32R = mybir.dt.float32r
BF16 = mybir.dt.bfloat16
AF = mybir.ActivationFunctionType
N_CORES = 8
B, T, C = 2, 2048, 1024
H, HS = 16, 64
FF = 4 * C                # 4096
TL = (B * T) // N_CORES   # 512 local token rows per core
P = 128
LN_EPS = 1e-5
NEG = -1e9

_BUILT = None  # cache the compiled Bass module across calls


def _build(reps=1):
    nc = bacc.Bacc(None, target_bir_lowering=False, debug=False,
                   num_devices=N_CORES)

    # ---- external I/O (per core) ----
    x_loc = nc.declare_dram_parameter("x_loc", [TL, C], F32, isOutput=False)
    wq_p = nc.declare_dram_parameter("wq_p", [C, C], F32R, isOutput=False)
    wk_p = nc.declare_dram_parameter("wk_p", [C, C], F32R, isOutput=False)
    wv_p = nc.declare_dram_parameter("wv_p", [C, C], F32R, isOutput=False)
    bias_q = nc.declare_dram_parameter("bias_q", [P, 8], F32, isOutput=False)
    bias_k = nc.declare_dram_parameter("bias_k", [P, 8], F32, isOutput=False)
    bias_v = nc.declare_dram_parameter("bias_v", [1, C], F32, isOutput=False)
    w_proj = nc.declare_dram_parameter("w_proj", [C, C], BF16, isOutput=False)
    b_proj = nc.declare_dram_parameter("b_proj", [1, C], F32, isOutput=False)
    w1_t = nc.declare_dram_parameter("w1_t", [FF, C], F32R, isOutput=False)
    bias_ff1 = nc.declare_dram_parameter("bias_ff1", [P, 32], F32,
                                         isOutput=False)
    w2 = nc.declare_dram_parameter("w2", [FF, C], F32R, isOutput=False)
    b2 = nc.declare_dram_parameter("b2", [1, C], F32, isOutput=False)
    mask_t = nc.declare_dram_parameter("mask_t", [P, P], F32, isOutput=False)
    ident = nc.declare_dram_parameter("ident", [P, P], F32, isOutput=False)
    out = nc.declare_dram_parameter("out", [TL, C], F32, isOutput=True)

    # ---- internal DRAM for collectives ----
    # QT/KT: flat [8*128, 512]; A2A slot j (rows 128j..) = head-pair j of my
    # local tokens. After A2A, rows 128r.. = my head-pair for core r's tokens.
    qk_in = nc.dram_tensor("qk_in", [8 * 2 * P, TL], BF16)
    qk_mine = nc.dram_tensor("qk_mine", [8 * 2 * P, TL], BF16)
    # V: flat [8*512, 128]; slot j = column block (head-pair j) of local V.
    v_in = nc.dram_tensor("v_in", [8 * TL, P], BF16)
    v_mine = nc.dram_tensor("v_mine", [8 * TL, P], BF16)
    # attention out: slot j = my heads' output for core j's token rows.
    attn_in = nc.dram_tensor("attn_in", [8 * TL, P], BF16)
    attn_mine = nc.dram_tensor("attn_mine", [8 * TL, P], BF16)

    groups = [list(range(N_CORES))]

    from contextlib import ExitStack
    with tile.TileContext(nc) as tc, ExitStack() as stk:
        const = stk.enter_context(tc.tile_pool(name="const", bufs=1))
        ident_sb = const.tile([P, P], F32, tag="ident", name="ident_sb")
        identb_sb = const.tile([P, P], BF16, tag="identb", name="identb_sb")
        mask_sb = const.tile([P, P], F32, tag="mask", name="mask_sb")
        eps_sb = const.tile([P, 1], F32, tag="eps", name="eps_sb")
        ones_sb = const.tile([1, P], F32, tag="ones", name="ones_sb")
        ones2_sb = const.tile([P, 2, 1], BF16, tag="ones2", name="ones2_sb")
        bq_sb = const.tile([P, 8], F32, tag="bq", name="bq_sb")
        bk_sb = const.tile([P, 8], F32, tag="bk", name="bk_sb")
        bv_sb = const.tile([P, C], F32, tag="bv", name="bv_sb")
        bproj_sb = const.tile([P, C], F32, tag="bproj", name="bproj_sb")
        b2_sb = const.tile([P, C], F32, tag="b2c", name="b2_sb")
        bff1_sb = const.tile([P, 32], F32, tag="bff1", name="bff1_sb")
        wps_c = [[const.tile([P, 512], BF16, tag=f"wpc{n}_{p}",
                             name=f"wpc{n}_{p}") for p in range(8)]
                 for n in range(2)]
        for n in range(2):
            for p in range(8):
                nc.sync.dma_start(
                    out=wps_c[n][p][:, :],
                    in_=w_proj[p * P:(p + 1) * P, n * 512:(n + 1) * 512])
        nc.sync.dma_start(out=ident_sb[:, :], in_=ident[:, :])
        nc.vector.tensor_copy(identb_sb[:, :], ident_sb[:, :])
        nc.sync.dma_start(out=mask_sb[:, :], in_=mask_t[:, :])
        nc.sync.dma_start(out=bq_sb[:, :], in_=bias_q[:, :])
        nc.sync.dma_start(out=bk_sb[:, :], in_=bias_k[:, :])
        nc.sync.dma_start(out=bv_sb[:, :], in_=bias_v[:, :].to_broadcast([P, C]))
        nc.sync.dma_start(out=bproj_sb[:, :], in_=b_proj[:, :].to_broadcast([P, C]))
        nc.sync.dma_start(out=b2_sb[:, :], in_=b2[:, :].to_broadcast([P, C]))
        nc.sync.dma_start(out=bff1_sb[:, :], in_=bias_ff1[:, :])
        nc.vector.memset(eps_sb[:, :], LN_EPS)
        nc.vector.memset(ones_sb[:, :], 1.0)
        nc.vector.memset(ones2_sb[:, :, :], 1.0)

        # x / x2 tiles stay resident for the two residual adds.
        xres = stk.enter_context(tc.tile_pool(name="xres", bufs=1))
        x_tiles = [xres.tile([P, C], F32, tag=f"x{t}", name=f"x{t}")
                   for t in range(4)]
        x2_tiles = [xres.tile([P, C], F32, tag=f"x2_{t}", name=f"x2_{t}")
                    for t in range(4)]

        def layernorm_tiles(src_tiles, dst_pool, dst_tag):
            """LN over the free dim (1024) of 4 [128, 1024] tiles."""
            hats = []
            for t in range(4):
                xt = src_tiles[t]
                stats = dst_pool.tile([P, 2, nc.vector.BN_STATS_DIM], F32,
                                      tag=f"lns{dst_tag}{t}",
                                      name=f"stats_{dst_tag}{t}")
                xg = xt[:, :].rearrange("p (s d) -> p s d", s=2)
                for s in range(2):
                    nc.vector.bn_stats(out=stats[:, s, :], in_=xg[:, s, :])
                mv = dst_pool.tile([P, nc.vector.BN_AGGR_DIM], F32,
                                   tag=f"lnm{dst_tag}{t}",
                                   name=f"mv_{dst_tag}{t}")
                nc.vector.bn_aggr(out=mv[:, :], in_=stats[:, :, :])
                nc.scalar.activation(out=mv[:, 1:2], in_=mv[:, 1:2],
                                     func=AF.Sqrt, bias=eps_sb[:, :])
                nc.vector.reciprocal(out=mv[:, 1:2], in_=mv[:, 1:2])
                # negated scaled mean for the ACT Identity pass below
                nmu = dst_pool.tile([P, 1], F32, tag=f"lnn{dst_tag}{t}",
                                    name=f"nmu_{dst_tag}{t}")
                nc.vector.tensor_tensor(out=nmu[:, :], in0=mv[:, 0:1],
                                        in1=mv[:, 1:2],
                                        op=mybir.AluOpType.mult)
                nc.vector.tensor_scalar_mul(out=nmu[:, :], in0=nmu[:, :],
                                            scalar1=-1.0)
                hat = dst_pool.tile([P, C], F32, tag=f"{dst_tag}{t}",
                                    name=f"{dst_tag}{t}")
                nc.scalar.activation(out=hat[:, :], in_=xt[:, :],
                                     func=AF.Identity, bias=nmu[:, :],
                                     scale=mv[:, 1:2])
                hats.append(hat)
            return hats

        def transpose_to(hats, dst_tiles, psum_pool, tagp):
            """4x [128, 1024] token-major -> 8x [128, 512] channel-major."""
            for cc in range(8):
                for t in range(4):
                    pt = psum_pool.tile([P, P], F32, tag=tagp,
                                        name=f"tr_{tagp}{cc}_{t}")
                    nc.tensor.transpose(pt[:, :],
                                        hats[t][:, cc * P:(cc + 1) * P],
                                        ident_sb[:, :])
                    nc.vector.tensor_copy(
                        dst_tiles[cc][:, t * P:(t + 1) * P], pt[:, :])

        def run_pipeline():
            # ================= Phase A+B: LN1, h^T, QKV =================
            with tc.tile_pool(name="pa", bufs=1) as pa, \
                 tc.tile_pool(name="pa_w", bufs=4) as pa_w, \
                 tc.tile_pool(name="pa_tr", bufs=2, space="PSUM") as pa_tr, \
                 tc.tile_pool(name="pa_mm", bufs=2, space="PSUM") as pa_mm:
                for t in range(4):
                    nc.sync.dma_start(out=x_tiles[t][:, :],
                                      in_=x_loc[t * P:(t + 1) * P, :])
                hats = layernorm_tiles(x_tiles, pa, "hat")
                hT = [pa.tile([P, TL], F32R, tag=f"hT{cc}", name=f"hT{cc}")
                      for cc in range(8)]
                transpose_to(hats, hT, pa_tr, "trh")

                # QT/KT[kb][:, t] = sum_c W[c, 128kb+*] hT[c, t]   (+ bias)
                for (w_dram, b_sb, off, qn) in ((wk_p, bk_sb, P, "k"),
                                                (wq_p, bq_sb, 0, "q")):
                    for kb in range(8):
                        wt = pa_w.tile([P, C], F32R, tag="wqk",
                                       name=f"wl_{qn}{kb}")
                        nc.sync.dma_start(
                            out=wt[:, :],
                            in_=w_dram[kb * P:(kb + 1) * P, :])
                        ps = pa_mm.tile([P, TL], F32, tag="qk_ps",
                                        name=f"qk_ps_{qn}{kb}")
                        for cc in range(8):
                            nc.tensor.matmul(ps[:, :],
                                             (wt[:, cc * P:(cc + 1) * P]),
                                             (hT[cc][:, :]),
                                             start=(cc == 0), stop=(cc == 7))
                        ev = pa_w.tile([P, TL], BF16, tag="qk_ev",
                                       name=f"qk_ev_{qn}{kb}")
                        nc.vector.tensor_scalar_add(out=ev[:, :], in0=ps[:, :],
                                                    scalar1=b_sb[:, kb:kb + 1])
                        nc.sync.dma_start(
                            out=qk_in[kb * 2 * P + off:kb * 2 * P + off + P, :],
                            in_=ev[:, :])

                # V[t, :] = sum_c hT[c, t] wv[c, :]  (+ bias via ones row)
                v_i3 = v_in.rearrange("(j t) d -> j t d", j=8)
                for n in range(2):
                    wvs = [pa.tile([P, 512], F32R, tag=f"wv{cc}",
                                   name=f"wv{n}_{cc}") for cc in range(8)]
                    for cc in range(8):
                        nc.sync.dma_start(
                            out=wvs[cc][:, :],
                            in_=wv_p[cc * P:(cc + 1) * P, n * 512:(n + 1) * 512])
                    for t in range(4):
                        ps = pa_mm.tile([P, 512], F32, tag="v_ps",
                                        name=f"v_ps{n}_{t}")
                        for cc in range(8):
                            nc.tensor.matmul(ps[:, :],
                                             (hT[cc][:, t * P:(t + 1) * P]),
                                             (wvs[cc][:, :]),
                                             start=(cc == 0), stop=(cc == 7))
                        ev = pa_w.tile([P, 512], BF16, tag="v_ev",
                                       name=f"v_ev{n}_{t}")
                        nc.vector.tensor_add(out=ev[:, :], in0=ps[:, :],
                                             in1=bv_sb[:, n * 512:(n + 1) * 512])
                        # scatter the 4 pair-column blocks into their A2A slots
                        for j in range(4):
                            nc.sync.dma_start(
                                out=v_i3[n * 4 + j, t * P:(t + 1) * P, :],
                                in_=ev[:, j * P:(j + 1) * P])

            # ================= Phase C: A2A QKV =================
            nc.gpsimd.collective_compute("AllToAll", mybir.AluOpType.bypass,
                                         replica_groups=groups,
                                         ins=[qk_in[:, :]], outs=[qk_mine[:, :]])
            nc.gpsimd.collective_compute("AllToAll", mybir.AluOpType.bypass,
                                         replica_groups=groups,
                                         ins=[v_in[:, :]], outs=[v_mine[:, :]])

            # ================= Phase D: attention (my 2 heads, full T) ==========
            qk_m4 = qk_mine[:, :].rearrange("(r s p) t -> r s p t", r=8, s=2)
            v_m3 = v_mine[:, :].rearrange("(r t) d -> r t d", r=8)
            attn_i3 = attn_in[:, :].rearrange("(r t) d -> r t d", r=8)

            with tc.tile_pool(name="att_kv", bufs=2) as att_kv, \
                 tc.tile_pool(name="att_sb", bufs=4) as att_sb, \
                 tc.tile_pool(name="att_sc", bufs=2, space="PSUM") as att_sc, \
                 tc.tile_pool(name="att_tr", bufs=2, space="PSUM") as att_tr, \
                 tc.tile_pool(name="att_o", bufs=2, space="PSUM") as att_o:
                for b in range(B):
                    kt_all = att_kv.tile([P, 16, P], BF16, tag="kt_all",
                                         name=f"kt_all{b}")
                    v_all = att_kv.tile([P, 16, 2, 65], BF16, tag="v_all",
                                        name=f"v_all{b}")
                    for sc in range(16):
                        g = b * 16 + sc
                        r, o = g // 4, (g % 4) * P
                        nc.sync.dma_start(out=kt_all[:, sc, :],
                                          in_=qk_m4[r, 1, :, o:o + P])
                        nc.sync.dma_start(
                            out=v_all[:, sc, :, 0:64],
                            in_=v_m3[r, o:o + P, :].rearrange(
                                "t (h d) -> t h d", h=2))
                        nc.vector.tensor_copy(v_all[:, sc, :, 64:65],
                                              ones2_sb[:, :, :])
                    for qg in range(4):
                        # q-group = 512 queries = exactly core (b*4+qg)'s tokens
                        qt_t = att_sb.tile([P, 512], BF16, tag="qt_t",
                                           name=f"qt{b}_{qg}")
                        nc.sync.dma_start(out=qt_t[:, :],
                                          in_=qk_m4[b * 4 + qg, 0, :, :])
                        att_grp = att_sb.tile([P, 4, P], BF16, tag="att_grp",
                                              name=f"ag{b}_{qg}")
                        nblocks = 4 * qg + 4
                        o_ps = [att_o.tile([65, 512], F32, tag="outT",
                                           name=f"oT{b}_{qg}_{hh}")
                                for hh in range(2)]
                        for sb in range(nblocks):
                            j = max(0, sb - 4 * qg)  # first valid q-subchunk
                            q0 = j * P
                            sc_ps = att_sc.tile([P, 2, 512], F32, tag="sc_ps",
                                                name=f"sc{b}_{qg}_{sb}")
                            for h in range(2):
                                nc.tensor.matmul(
                                    sc_ps[:, h, q0:512],
                                    (kt_all[h * 64:(h + 1) * 64, sb, :]),
                                    (qt_t[h * 64:(h + 1) * 64, q0:512]),
                                    start=True, stop=True)
                            if sb >= 4 * qg:  # diagonal sub-block needs mask
                                for h in range(2):
                                    nc.vector.tensor_add(
                                        sc_ps[:, h, q0:q0 + P],
                                        sc_ps[:, h, q0:q0 + P], mask_sb[:, :])
                            ex = att_sb.tile([P, 2, 512], BF16, tag="ex",
                                             name=f"ex{b}_{qg}_{sb}")
                            nc.scalar.activation(out=ex[:, :, q0:512],
                                                 in_=sc_ps[:, :, q0:512],
                                                 func=AF.Exp,
                                                 scale=float(C) ** -0.5)
                            for h in range(2):
                                nc.tensor.matmul(o_ps[h][:, q0:512],
                                                 (v_all[:, sb, h, :]),
                                                 (ex[:, h, q0:512]),
                                                 start=(sb == 0),
                                                 stop=(sb == nblocks - 1))
                        for h in range(2):
                            oT_sb = att_sb.tile([65, 512], F32, tag="oT_sb",
                                                name=f"oTs{b}_{qg}_{h}")
                            nc.vector.tensor_copy(oT_sb[:, :], o_ps[h][:, :])
                            for j in range(4):
                                tr_ps = att_tr.tile([P, 65], F32, tag="tr_ps",
                                                    name=f"trp{b}_{qg}_{h}_{j}")
                                nc.tensor.transpose(
                                    tr_ps[:, :], oT_sb[:, j * P:(j + 1) * P],
                                    ident_sb[0:65, 0:65])
                                rec = att_sb.tile([P, 1], F32, tag="rec",
                                                  name=f"rec{b}_{qg}_{h}_{j}")
                                nc.vector.reciprocal(rec[:, :], tr_ps[:, 64:65])
                                nc.vector.tensor_scalar_mul(
                                    out=att_grp[:, j, h * 64:(h + 1) * 64],
                                    in0=tr_ps[:, 0:64], scalar1=rec[:, :])
                        for j in range(4):
                            nc.sync.dma_start(
                                out=attn_i3[b * 4 + qg, j * P:(j + 1) * P, :],
                                in_=att_grp[:, j, :])

            # ================= Phase E: A2A attention out =================
            nc.gpsimd.collective_compute("AllToAll", mybir.AluOpType.bypass,
                                         replica_groups=groups,
                                         ins=[attn_in[:, :]],
                                         outs=[attn_mine[:, :]])
            attn_m3 = attn_mine[:, :].rearrange("(r t) d -> r t d", r=8)

            # ================= Phase F: proj + residual =================
            with tc.tile_pool(name="pr_sb", bufs=1) as pr_sb, \
                 tc.tile_pool(name="pr_w", bufs=3) as pr_w, \
                 tc.tile_pool(name="pr_tr", bufs=2, space="PSUM") as pr_tr, \
                 tc.tile_pool(name="pr_mm", bufs=2, space="PSUM") as pr_mm:
                attnT = [pr_sb.tile([P, TL], BF16, tag=f"aT{p}", name=f"aT{p}")
                         for p in range(8)]
                for p in range(8):
                    for t in range(4):
                        at = pr_w.tile([P, P], BF16, tag="at_ld",
                                       name=f"atl{p}_{t}")
                        nc.sync.dma_start(out=at[:, :],
                                          in_=attn_m3[p, t * P:(t + 1) * P, :])
                        pt = pr_tr.tile([P, P], BF16, tag="at_tr",
                                        name=f"attr{p}_{t}")
                        nc.tensor.transpose(pt[:, :], at[:, :],
                                            identb_sb[:, :])
                        nc.scalar.copy(attnT[p][:, t * P:(t + 1) * P],
                                       pt[:, :])
                for n in range(2):
                    wps = wps_c[n]
                    for t in range(4):
                        ps = pr_mm.tile([P, 512], F32, tag="pr_ps",
                                        name=f"prps{n}_{t}")
                        for p in range(8):
                            nc.tensor.matmul(ps[:, :],
                                             (attnT[p][:, t * P:(t + 1) * P]),
                                             (wps[p][:, :]),
                                             start=(p == 0), stop=(p == 7))
                        nc.vector.tensor_add(
                            out=x2_tiles[t][:, n * 512:(n + 1) * 512],
                            in0=ps[:, :],
                            in1=x_tiles[t][:, n * 512:(n + 1) * 512])
                        nc.vector.tensor_add(
                            out=x2_tiles[t][:, n * 512:(n + 1) * 512],
                            in0=x2_tiles[t][:, n * 512:(n + 1) * 512],
                            in1=bproj_sb[:, n * 512:(n + 1) * 512])

            # ================= Phase G+H+I: LN2 + FFN + residual ==============
            with tc.tile_pool(name="ff_sb", bufs=1) as ff_sb, \
                 tc.tile_pool(name="ff_w", bufs=6) as ff_w, \
                 tc.tile_pool(name="ff_tr", bufs=2, space="PSUM") as ff_tr, \
                 tc.tile_pool(name="ff1_ps", bufs=2, space="PSUM") as ff1_psp, \
                 tc.tile_pool(name="ff2_ps", bufs=4, space="PSUM") as ff2_psp:
                hats2 = layernorm_tiles(x2_tiles, ff_sb, "hat2")
                h2T = [ff_sb.tile([P, TL], F32R, tag=f"h2T{cc}", name=f"h2T{cc}")
                       for cc in range(8)]
                transpose_to(hats2, h2T, ff_tr, "trh2")

                # FFN1 (transposed): ffT[k, t] = relu(sum_c w1[c, k] h2T[c, t] + b)
                ffT = [ff_sb.tile([P, 4, TL], F32R, tag=f"ffT{i}", name=f"ffT{i}")
                       for i in range(8)]
                for kb in range(32):
                    w1t = ff_w.tile([P, C], F32R, tag="w1_ld", name=f"w1l{kb}")
                    nc.sync.dma_start(
                        out=w1t[:, :],
                        in_=w1_t[kb * P:(kb + 1) * P, :])
                    ps = ff1_psp.tile([P, TL], F32, tag="ff1_ps",
                                      name=f"ff1ps{kb}")
                    for cc in range(8):
                        nc.tensor.matmul(ps[:, :],
                                         (w1t[:, cc * P:(cc + 1) * P]),
                                         (h2T[cc][:, :]),
                                         start=(cc == 0), stop=(cc == 7))
                    nc.scalar.activation(out=ffT[kb // 4][:, kb % 4, :],
                                         in_=ps[:, :], func=AF.Relu,
                                         bias=bff1_sb[:, kb:kb + 1])

                # FFN2: out[t, c] = sum_k ffT[k, t] w2[k, c] + b2 + x2
                for n in range(2):
                    pss = [ff2_psp.tile([P, 512], F32, tag="ff2_ps",
                                        name=f"ff2ps{n}_{t}") for t in range(4)]
                    for kb in range(32):
                        w2t = ff_w.tile([P, 512], F32R, tag="w2_ld",
                                        name=f"w2l{n}_{kb}")
                        nc.sync.dma_start(
                            out=w2t[:, :],
                            in_=w2[kb * P:(kb + 1) * P, n * 512:(n + 1) * 512])
                        for t in range(4):
                            nc.tensor.matmul(
                                pss[t][:, :],
                                (ffT[kb // 4][:, kb % 4, t * P:(t + 1) * P]),
                                (w2t[:, :]),
                                start=(kb == 0), stop=(kb == 31))
                    for t in range(4):
                        yt = ff_w.tile([P, 512], F32, tag="y_ev",
                                       name=f"y{n}_{t}")
                        nc.vector.tensor_add(
                            out=yt[:, :], in0=pss[t][:, :],
                            in1=x2_tiles[t][:, n * 512:(n + 1) * 512])
                        nc.vector.tensor_add(
                            out=yt[:, :], in0=yt[:, :],
                            in1=b2_sb[:, n * 512:(n + 1) * 512])
                        nc.sync.dma_start(
                            out=out[t * P:(t + 1) * P, n * 512:(n + 1) * 512],
                            in_=yt[:, :])

        for _rep in range(reps):
            run_pipeline()

    nc.compile()
    return nc


def _block_t(w, rb, cb):
    """out[kb*128+p, cc*128+m] = w[cc*128+p, kb*128+m] (block transpose)."""
    return np.ascontiguousarray(
        w.reshape(rb, 128, cb, 128).transpose(2, 1, 0, 3).reshape(cb * 128,
                                                                  rb * 128))


def _prep_inputs(inputs):
    x = np.asarray(inputs["x"], dtype=np.float32)
    wq = np.asarray(inputs["wq"], dtype=np.float32)
    wk = np.asarray(inputs["wk"], dtype=np.float32)
    wv = np.asarray(inputs["wv"], dtype=np.float32)
    w_proj = np.asarray(inputs["w_proj"], dtype=np.float32)
    b_proj = np.asarray(inputs["b_proj"], dtype=np.float32)
    w1 = np.asarray(inputs["w1"], dtype=np.float32)
    b1 = np.asarray(inputs["b1"], dtype=np.float32)
    w2 = np.asarray(inputs["w2"], dtype=np.float32)
    b2 = np.asarray(inputs["b2"], dtype=np.float32)
    ln1_g = np.asarray(inputs["ln1_g"], dtype=np.float32)
    ln1_b = np.asarray(inputs["ln1_b"], dtype=np.float32)
    ln2_g = np.asarray(inputs["ln2_g"], dtype=np.float32)
    ln2_b = np.asarray(inputs["ln2_b"], dtype=np.float32)

    xf = x.reshape(B * T, C)
    Wq = wq.transpose(1, 0, 2).reshape(C, C)   # [c, h*HS + d]
    Wk = wk.transpose(1, 0, 2).reshape(C, C)
    Wv = wv.transpose(1, 0, 2).reshape(C, C)

    s = np.arange(P)[:, None]
    q = np.arange(P)[None, :]
    common = {
        "wq_p": _block_t(ln1_g[:, None] * Wq, 8, 8),
        "wk_p": _block_t(ln1_g[:, None] * Wk, 8, 8),
        "wv_p": np.ascontiguousarray(ln1_g[:, None] * Wv),
        "bias_q": np.ascontiguousarray((ln1_b @ Wq).reshape(8, P).T),
        "bias_k": np.ascontiguousarray((ln1_b @ Wk).reshape(8, P).T),
        "bias_v": (ln1_b @ Wv).reshape(1, C),
        "w_proj": w_proj.astype(ml_dtypes.bfloat16), "b_proj": b_proj.reshape(1, C),
        "w1_t": _block_t(ln2_g[:, None] * w1, 8, 32),
        "bias_ff1": np.ascontiguousarray((b1 + ln2_b @ w1).reshape(32, P).T),
        "w2": w2, "b2": b2.reshape(1, C),
        "mask_t": np.where(s <= q, 0.0, NEG).astype(np.float32),
        "ident": np.eye(P, dtype=np.float32),
    }
    common = {k: (np.ascontiguousarray(v) if k == "w_proj"
                  else np.ascontiguousarray(v, dtype=np.float32))
              for k, v in common.items()}
    in_maps = []
    for i in range(N_CORES):
        m = dict(common)
        m["x_loc"] = np.ascontiguousarray(xf[i * TL:(i + 1) * TL, :])
        in_maps.append(m)
    return in_maps


def kernel(**inputs):
    global _BUILT
    if _BUILT is None:
        _BUILT = _build()
    in_maps = _prep_inputs(inputs)
    res = run_bass_kernel_spmd(_BUILT, in_maps, core_ids=list(range(N_CORES)))
    outf = np.concatenate([res.results[i]["out"] for i in range(N_CORES)],
                          axis=0)
    return outf.reshape(B, T, C).astype(np.float32)



# revision 2
# speedup vs baseline: 1.2481x; 1.2481x over previous
"""Fused transformer block (LN -> 16-head causal attention -> proj -> residual
-> LN -> FFN -> residual) on 8 TRN2 NeuronCores.

Sharding: tokens are sharded across cores for LN/QKV/proj/FFN (512 rows of the
flattened [4096, 1024] each); attention is head-sharded (2 heads per core) so
every core runs an identical SPMD program over the full causal triangle.
AllToAll collectives redistribute Q^T/K^T/V (token-sharded -> head-sharded)
and the attention output (head-sharded -> token-sharded).

All matmuls keep operands pre-transposed so the contraction dim is always the
SBUF partition dim: h and h2 are transposed on-chip via the PE; Q/K are
produced directly in [head_dim, token] layout; attention scores are computed
transposed ([key, query]) so the softmax-weighted V accumulation is a plain
PSUM matmul chain whose appended ones-column yields the softmax denominator.
LayerNorm gains are folded into the following weight matrices host-side;
LayerNorm biases become per-output-channel biases applied on PSUM eviction
or via rank-1 ones-row matmuls.

Weights are bf16 and distributed one of two ways (WEIGHT_MODE):
  "const"  — embedded in the NEFF as Const DRAM tensors, DMA'd to HBM once at
             model load. Per-execution host->device traffic is then only the
             activation input x (the weights never re-cross the slow axon
             host link).
  "gather" — staged as per-core 1/8 row-shards (each weight byte crosses the
             host link once, not 8x) and AllGathered core-to-core over
             NeuronLink into internal DRAM at kernel start.
"""

import hashlib
import os

import ml_dtypes
import numpy as np

import concourse.bass as bass  # noqa: F401  (AP helpers via handles)
import concourse.mybir as mybir
import concourse.tile as tile
from concourse import bacc
from concourse.bass_utils import run_bass_kernel_spmd

F32 = mybir.dt.float32
BF16 = mybir.dt.bfloat16
AF = mybir.ActivationFunctionType
N_CORES = 8
B, T, C = 2, 2048, 1024
H, HS = 16, 64
FF = 4 * C                # 4096
TL = (B * T) // N_CORES   # 512 local token rows per core
P = 128
LN_EPS = 1e-5
NEG = -1e9
BF = ml_dtypes.bfloat16

WEIGHT_MODE = os.environ.get("KERNEL_WEIGHT_MODE", "const")

_BUILT = None       # cache the compiled Bass module across calls
_BUILT_KEY = None   # fingerprint of the weights baked into _BUILT

# names/shapes of the weight-side tensors (everything except x), in the
# row-sharded layout used by both modes. All shard cleanly along dim 0.
_WEIGHT_SPECS = [
    # (name, rows, cols, np dtype)
    ("wq_p", C, C, BF), ("wk_p", C, C, BF), ("wv_p", C, C, BF),
    ("w_proj", C, C, BF), ("w1_t", FF, C, BF), ("w2", FF, C, BF),
]
_SMALL_SPECS = [
    ("bias_q", P, 8, np.float32), ("bias_k", P, 8, np.float32),
    ("bias_v", 1, C, np.float32), ("b_proj", 1, C, np.float32),
    ("bias_ff1", P, 32, np.float32), ("b2", 1, C, np.float32),
    ("mask_t", P, P, np.float32), ("ident", P, P, np.float32),
]


def _build(common, reps=1):
    """common: dict name -> full prepped numpy array (see _prep_common)."""
    nc = bacc.Bacc(None, target_bir_lowering=False, debug=False,
                   num_devices=N_CORES)

    # ---- external I/O (per core) ----
    x_loc = nc.declare_dram_parameter("x_loc", [TL, C], F32, isOutput=False)
    out = nc.declare_dram_parameter("out", [TL, C], F32, isOutput=True)

    dram = {}
    gathers = []  # (shard_param, gathered_tensor) pairs for gather mode
    if WEIGHT_MODE == "const":
        for name, arr in common.items():
            dram[name] = nc.inline_tensor(np.ascontiguousarray(arr),
                                          name=name)
    else:
        for name, rows, cols, dt in _WEIGHT_SPECS:
            bdt = BF16 if dt == BF else F32
            sh = nc.declare_dram_parameter(f"{name}_s", [rows // N_CORES, cols],
                                           bdt, isOutput=False)
            g = nc.dram_tensor(name, [rows, cols], bdt)
            dram[name] = g
            gathers.append((sh, g))
        for name, rows, cols, dt in _SMALL_SPECS:
            dram[name] = nc.inline_tensor(
                np.ascontiguousarray(common[name]), name=name)

    wq_p, wk_p, wv_p = dram["wq_p"], dram["wk_p"], dram["wv_p"]
    w_proj, w1_t, w2 = dram["w_proj"], dram["w1_t"], dram["w2"]
    bias_q, bias_k, bias_v = dram["bias_q"], dram["bias_k"], dram["bias_v"]
    b_proj, bias_ff1, b2 = dram["b_proj"], dram["bias_ff1"], dram["b2"]
    mask_t, ident = dram["mask_t"], dram["ident"]

    # ---- internal DRAM for collectives ----
    # QT/KT: flat [8*128, 512]; A2A slot j (rows 128j..) = head-pair j of my
    # local tokens. After A2A, rows 128r.. = my head-pair for core r's tokens.
    qk_in = nc.dram_tensor("qk_in", [8 * 2 * P, TL], BF16)
    qk_mine = nc.dram_tensor("qk_mine", [8 * 2 * P, TL], BF16)
    # V: flat [8*512, 128]; slot j = column block (head-pair j) of local V.
    v_in = nc.dram_tensor("v_in", [8 * TL, P], BF16)
    v_mine = nc.dram_tensor("v_mine", [8 * TL, P], BF16)
    # attention out: slot j = my heads' output for core j's token rows.
    attn_in = nc.dram_tensor("attn_in", [8 * TL, P], BF16)
    attn_mine = nc.dram_tensor("attn_mine", [8 * TL, P], BF16)

    groups = [list(range(N_CORES))]

    from contextlib import ExitStack
    with tile.TileContext(nc) as tc, ExitStack() as stk:
        if WEIGHT_MODE != "const":
            for sh, g in gathers:
                nc.gpsimd.collective_compute(
                    "AllGather", mybir.AluOpType.bypass,
                    replica_groups=groups, ins=[sh[:, :]], outs=[g[:, :]])

        const = stk.enter_context(tc.tile_pool(name="const", bufs=1))
        ident_sb = const.tile([P, P], F32, tag="ident", name="ident_sb")
        identb_sb = const.tile([P, P], BF16, tag="identb", name="identb_sb")
        mask_sb = const.tile([P, P], F32, tag="mask", name="mask_sb")
        eps_sb = const.tile([P, 1], F32, tag="eps", name="eps_sb")
        ones_sb = const.tile([1, P], F32, tag="ones", name="ones_sb")
        ones2_sb = const.tile([P, 2, 1], BF16, tag="ones2", name="ones2_sb")
        bq_sb = const.tile([P, 8], F32, tag="bq", name="bq_sb")
        bk_sb = const.tile([P, 8], F32, tag="bk", name="bk_sb")
        bv_sb = const.tile([P, C], F32, tag="bv", name="bv_sb")
        bproj_sb = const.tile([P, C], F32, tag="bproj", name="bproj_sb")
        b2_sb = const.tile([P, C], F32, tag="b2c", name="b2_sb")
        bff1_sb = const.tile([P, 32], F32, tag="bff1", name="bff1_sb")
        wps_c = [[const.tile([P, 512], BF16, tag=f"wpc{n}_{p}",
                             name=f"wpc{n}_{p}") for p in range(8)]
                 for n in range(2)]
        for n in range(2):
            for p in range(8):
                nc.sync.dma_start(
                    out=wps_c[n][p][:, :],
                    in_=w_proj[p * P:(p + 1) * P, n * 512:(n + 1) * 512])
        nc.sync.dma_start(out=ident_sb[:, :], in_=ident[:, :])
        nc.vector.tensor_copy(identb_sb[:, :], ident_sb[:, :])
        nc.sync.dma_start(out=mask_sb[:, :], in_=mask_t[:, :])
        nc.sync.dma_start(out=bq_sb[:, :], in_=bias_q[:, :])
        nc.sync.dma_start(out=bk_sb[:, :], in_=bias_k[:, :])
        nc.sync.dma_start(out=bv_sb[:, :], in_=bias_v[:, :].to_broadcast([P, C]))
        nc.sync.dma_start(out=bproj_sb[:, :], in_=b_proj[:, :].to_broadcast([P, C]))
        nc.sync.dma_start(out=b2_sb[:, :], in_=b2[:, :].to_broadcast([P, C]))
        nc.sync.dma_start(out=bff1_sb[:, :], in_=bias_ff1[:, :])
        nc.vector.memset(eps_sb[:, :], LN_EPS)
        nc.vector.memset(ones_sb[:, :], 1.0)
        nc.vector.memset(ones2_sb[:, :, :], 1.0)

        # x / x2 tiles stay resident for the two residual adds.
        xres = stk.enter_context(tc.tile_pool(name="xres", bufs=1))
        x_tiles = [xres.tile([P, C], F32, tag=f"x{t}", name=f"x{t}")
                   for t in range(4)]
        x2_tiles = [xres.tile([P, C], F32, tag=f"x2_{t}", name=f"x2_{t}")
                    for t in range(4)]

        def layernorm_tiles(src_tiles, dst_pool, dst_tag):
            """LN over the free dim (1024) of 4 [128, 1024] tiles."""
            hats = []
            for t in range(4):
                xt = src_tiles[t]
                stats = dst_pool.tile([P, 2, nc.vector.BN_STATS_DIM], F32,
                                      tag=f"lns{dst_tag}{t}",
                                      name=f"stats_{dst_tag}{t}")
                xg = xt[:, :].rearrange("p (s d) -> p s d", s=2)
                for s in range(2):
                    nc.vector.bn_stats(out=stats[:, s, :], in_=xg[:, s, :])
                mv = dst_pool.tile([P, nc.vector.BN_AGGR_DIM], F32,
                                   tag=f"lnm{dst_tag}{t}",
                                   name=f"mv_{dst_tag}{t}")
                nc.vector.bn_aggr(out=mv[:, :], in_=stats[:, :, :])
                nc.scalar.activation(out=mv[:, 1:2], in_=mv[:, 1:2],
                                     func=AF.Sqrt, bias=eps_sb[:, :])
                nc.vector.reciprocal(out=mv[:, 1:2], in_=mv[:, 1:2])
                # negated scaled mean for the ACT Identity pass below
                nmu = dst_pool.tile([P, 1], F32, tag=f"lnn{dst_tag}{t}",
                                    name=f"nmu_{dst_tag}{t}")
                nc.vector.tensor_tensor(out=nmu[:, :], in0=mv[:, 0:1],
                                        in1=mv[:, 1:2],
                                        op=mybir.AluOpType.mult)
                nc.vector.tensor_scalar_mul(out=nmu[:, :], in0=nmu[:, :],
                                            scalar1=-1.0)
                hat = dst_pool.tile([P, C], F32, tag=f"{dst_tag}{t}",
                                    name=f"{dst_tag}{t}")
                nc.scalar.activation(out=hat[:, :], in_=xt[:, :],
                                     func=AF.Identity, bias=nmu[:, :],
                                     scale=mv[:, 1:2])
                hats.append(hat)
            return hats

        def transpose_to(hats, dst_tiles, psum_pool, tagp):
            """4x [128, 1024] token-major -> 8x [128, 512] channel-major."""
            for cc in range(8):
                for t in range(4):
                    pt = psum_pool.tile([P, P], F32, tag=tagp,
                                        name=f"tr_{tagp}{cc}_{t}")
                    nc.tensor.transpose(pt[:, :],
                                        hats[t][:, cc * P:(cc + 1) * P],
                                        ident_sb[:, :])
                    nc.vector.tensor_copy(
                        dst_tiles[cc][:, t * P:(t + 1) * P], pt[:, :])

        def run_pipeline():
            # ================= Phase A+B: LN1, h^T, QKV =================
            with tc.tile_pool(name="pa", bufs=1) as pa, \
                 tc.tile_pool(name="pa_w", bufs=4) as pa_w, \
                 tc.tile_pool(name="pa_tr", bufs=2, space="PSUM") as pa_tr, \
                 tc.tile_pool(name="pa_mm", bufs=2, space="PSUM") as pa_mm:
                for t in range(4):
                    nc.sync.dma_start(out=x_tiles[t][:, :],
                                      in_=x_loc[t * P:(t + 1) * P, :])
                hats = layernorm_tiles(x_tiles, pa, "hat")
                hT = [pa.tile([P, TL], BF16, tag=f"hT{cc}", name=f"hT{cc}")
                      for cc in range(8)]
                transpose_to(hats, hT, pa_tr, "trh")

                # QT/KT[kb][:, t] = sum_c W[c, 128kb+*] hT[c, t]   (+ bias)
                for (w_dram, b_sb, off, qn) in ((wk_p, bk_sb, P, "k"),
                                                (wq_p, bq_sb, 0, "q")):
                    for kb in range(8):
                        wt = pa_w.tile([P, C], BF16, tag="wqk",
                                       name=f"wl_{qn}{kb}")
                        nc.sync.dma_start(
                            out=wt[:, :],
                            in_=w_dram[kb * P:(kb + 1) * P, :])
                        ps = pa_mm.tile([P, TL], F32, tag="qk_ps",
                                        name=f"qk_ps_{qn}{kb}")
                        for cc in range(8):
                            nc.tensor.matmul(ps[:, :],
                                             (wt[:, cc * P:(cc + 1) * P]),
                                             (hT[cc][:, :]),
                                             start=(cc == 0), stop=(cc == 7))
                        ev = pa_w.tile([P, TL], BF16, tag="qk_ev",
                                       name=f"qk_ev_{qn}{kb}")
                        nc.vector.tensor_scalar_add(out=ev[:, :], in0=ps[:, :],
                                                    scalar1=b_sb[:, kb:kb + 1])
                        nc.sync.dma_start(
                            out=qk_in[kb * 2 * P + off:kb * 2 * P + off + P, :],
                            in_=ev[:, :])

                # V[t, :] = sum_c hT[c, t] wv[c, :]  (+ bias via ones row)
                v_i3 = v_in.rearrange("(j t) d -> j t d", j=8)
                for n in range(2):
                    wvs = [pa.tile([P, 512], BF16, tag=f"wv{cc}",
                                   name=f"wv{n}_{cc}") for cc in range(8)]
                    for cc in range(8):
                        nc.sync.dma_start(
                            out=wvs[cc][:, :],
                            in_=wv_p[cc * P:(cc + 1) * P, n * 512:(n + 1) * 512])
                    for t in range(4):
                        ps = pa_mm.tile([P, 512], F32, tag="v_ps",
                                        name=f"v_ps{n}_{t}")
                        for cc in range(8):
                            nc.tensor.matmul(ps[:, :],
                                             (hT[cc][:, t * P:(t + 1) * P]),
                                             (wvs[cc][:, :]),
                                             start=(cc == 0), stop=(cc == 7))
                        ev = pa_w.tile([P, 512], BF16, tag="v_ev",
                                       name=f"v_ev{n}_{t}")
                        nc.vector.tensor_add(out=ev[:, :], in0=ps[:, :],
                                             in1=bv_sb[:, n * 512:(n + 1) * 512])
                        # scatter the 4 pair-column blocks into their A2A slots
                        for j in range(4):
                            nc.sync.dma_start(
                                out=v_i3[n * 4 + j, t * P:(t + 1) * P, :],
                                in_=ev[:, j * P:(j + 1) * P])

            # ================= Phase C: A2A QKV =================
            nc.gpsimd.collective_compute("AllToAll", mybir.AluOpType.bypass,
                                         replica_groups=groups,
                                         ins=[qk_in[:, :]], outs=[qk_mine[:, :]])
            nc.gpsimd.collective_compute("AllToAll", mybir.AluOpType.bypass,
                                         replica_groups=groups,
                                         ins=[v_in[:, :]], outs=[v_mine[:, :]])

            # ================= Phase D: attention (my 2 heads, full T) ==========
            qk_m4 = qk_mine[:, :].rearrange("(r s p) t -> r s p t", r=8, s=2)
            v_m3 = v_mine[:, :].rearrange("(r t) d -> r t d", r=8)
            attn_i3 = attn_in[:, :].rearrange("(r t) d -> r t d", r=8)

            with tc.tile_pool(name="att_kv", bufs=2) as att_kv, \
                 tc.tile_pool(name="att_sb", bufs=4) as att_sb, \
                 tc.tile_pool(name="att_sc", bufs=2, space="PSUM") as att_sc, \
                 tc.tile_pool(name="att_tr", bufs=2, space="PSUM") as att_tr, \
                 tc.tile_pool(name="att_o", bufs=2, space="PSUM") as att_o:
                for b in range(B):
                    kt_all = att_kv.tile([P, 16, P], BF16, tag="kt_all",
                                         name=f"kt_all{b}")
                    v_all = att_kv.tile([P, 16, 2, 65], BF16, tag="v_all",
                                        name=f"v_all{b}")
                    for sc in range(16):
                        g = b * 16 + sc
                        r, o = g // 4, (g % 4) * P
                        nc.sync.dma_start(out=kt_all[:, sc, :],
                                          in_=qk_m4[r, 1, :, o:o + P])
                        nc.sync.dma_start(
                            out=v_all[:, sc, :, 0:64],
                            in_=v_m3[r, o:o + P, :].rearrange(
                                "t (h d) -> t h d", h=2))
                        nc.vector.tensor_copy(v_all[:, sc, :, 64:65],
                                              ones2_sb[:, :, :])
                    for qg in range(4):
                        # q-group = 512 queries = exactly core (b*4+qg)'s tokens
                        qt_t = att_sb.tile([P, 512], BF16, tag="qt_t",
                                           name=f"qt{b}_{qg}")
                        nc.sync.dma_start(out=qt_t[:, :],
                                          in_=qk_m4[b * 4 + qg, 0, :, :])
                        att_grp = att_sb.tile([P, 4, P], BF16, tag="att_grp",
                                              name=f"ag{b}_{qg}")
                        nblocks = 4 * qg + 4
                        o_ps = [att_o.tile([65, 512], F32, tag="outT",
                                           name=f"oT{b}_{qg}_{hh}")
                                for hh in range(2)]
                        for sb in range(nblocks):
                            j = max(0, sb - 4 * qg)  # first valid q-subchunk
                            q0 = j * P
                            sc_ps = att_sc.tile([P, 2, 512], F32, tag="sc_ps",
                                                name=f"sc{b}_{qg}_{sb}")
                            for h in range(2):
                                nc.tensor.matmul(
                                    sc_ps[:, h, q0:512],
                                    (kt_all[h * 64:(h + 1) * 64, sb, :]),
                                    (qt_t[h * 64:(h + 1) * 64, q0:512]),
                                    start=True, stop=True)
                            if sb >= 4 * qg:  # diagonal sub-block needs mask
                                for h in range(2):
                                    nc.vector.tensor_add(
                                        sc_ps[:, h, q0:q0 + P],
                                        sc_ps[:, h, q0:q0 + P], mask_sb[:, :])
                            ex = att_sb.tile([P, 2, 512], BF16, tag="ex",
                                             name=f"ex{b}_{qg}_{sb}")
                            nc.scalar.activation(out=ex[:, :, q0:512],
                                                 in_=sc_ps[:, :, q0:512],
                                                 func=AF.Exp,
                                                 scale=float(C) ** -0.5)
                            for h in range(2):
                                nc.tensor.matmul(o_ps[h][:, q0:512],
                                                 (v_all[:, sb, h, :]),
                                                 (ex[:, h, q0:512]),
                                                 start=(sb == 0),
                                                 stop=(sb == nblocks - 1))
                        for h in range(2):
                            oT_sb = att_sb.tile([65, 512], F32, tag="oT_sb",
                                                name=f"oTs{b}_{qg}_{h}")
                            nc.vector.tensor_copy(oT_sb[:, :], o_ps[h][:, :])
                            for j in range(4):
                                tr_ps = att_tr.tile([P, 65], F32, tag="tr_ps",
                                                    name=f"trp{b}_{qg}_{h}_{j}")
                                nc.tensor.transpose(
                                    tr_ps[:, :], oT_sb[:, j * P:(j + 1) * P],
                                    ident_sb[0:65, 0:65])
                                rec = att_sb.tile([P, 1], F32, tag="rec",
                                                  name=f"rec{b}_{qg}_{h}_{j}")
                                nc.vector.reciprocal(rec[:, :], tr_ps[:, 64:65])
                                nc.vector.tensor_scalar_mul(
                                    out=att_grp[:, j, h * 64:(h + 1) * 64],
                                    in0=tr_ps[:, 0:64], scalar1=rec[:, :])
                        for j in range(4):
                            nc.sync.dma_start(
                                out=attn_i3[b * 4 + qg, j * P:(j + 1) * P, :],
                                in_=att_grp[:, j, :])

            # ================= Phase E: A2A attention out =================
            nc.gpsimd.collective_compute("AllToAll", mybir.AluOpType.bypass,
                                         replica_groups=groups,
                                         ins=[attn_in[:, :]],
                                         outs=[attn_mine[:, :]])
            attn_m3 = attn_mine[:, :].rearrange("(r t) d -> r t d", r=8)

            # ================= Phase F: proj + residual =================
            with tc.tile_pool(name="pr_sb", bufs=1) as pr_sb, \
                 tc.tile_pool(name="pr_w", bufs=3) as pr_w, \
                 tc.tile_pool(name="pr_tr", bufs=2, space="PSUM") as pr_tr, \
                 tc.tile_pool(name="pr_mm", bufs=2, space="PSUM") as pr_mm:
                attnT = [pr_sb.tile([P, TL], BF16, tag=f"aT{p}", name=f"aT{p}")
                         for p in range(8)]
                for p in range(8):
                    for t in range(4):
                        at = pr_w.tile([P, P], BF16, tag="at_ld",
                                       name=f"atl{p}_{t}")
                        nc.sync.dma_start(out=at[:, :],
                                          in_=attn_m3[p, t * P:(t + 1) * P, :])
                        pt = pr_tr.tile([P, P], BF16, tag="at_tr",
                                        name=f"attr{p}_{t}")
                        nc.tensor.transpose(pt[:, :], at[:, :],
                                            identb_sb[:, :])
                        nc.scalar.copy(attnT[p][:, t * P:(t + 1) * P],
                                       pt[:, :])
                for n in range(2):
                    wps = wps_c[n]
                    for t in range(4):
                        ps = pr_mm.tile([P, 512], F32, tag="pr_ps",
                                        name=f"prps{n}_{t}")
                        for p in range(8):
                            nc.tensor.matmul(ps[:, :],
                                             (attnT[p][:, t * P:(t + 1) * P]),
                                             (wps[p][:, :]),
                                             start=(p == 0), stop=(p == 7))
                        nc.vector.tensor_add(
                            out=x2_tiles[t][:, n * 512:(n + 1) * 512],
                            in0=ps[:, :],
                            in1=x_tiles[t][:, n * 512:(n + 1) * 512])
                        nc.vector.tensor_add(
                            out=x2_tiles[t][:, n * 512:(n + 1) * 512],
                            in0=x2_tiles[t][:, n * 512:(n + 1) * 512],
                            in1=bproj_sb[:, n * 512:(n + 1) * 512])

            # ================= Phase G+H+I: LN2 + FFN + residual ==============
            with tc.tile_pool(name="ff_sb", bufs=1) as ff_sb, \
                 tc.tile_pool(name="ff_w", bufs=6) as ff_w, \
                 tc.tile_pool(name="ff_tr", bufs=2, space="PSUM") as ff_tr, \
                 tc.tile_pool(name="ff1_ps", bufs=2, space="PSUM") as ff1_psp, \
                 tc.tile_pool(name="ff2_ps", bufs=4, space="PSUM") as ff2_psp:
                hats2 = layernorm_tiles(x2_tiles, ff_sb, "hat2")
                h2T = [ff_sb.tile([P, TL], BF16, tag=f"h2T{cc}", name=f"h2T{cc}")
                       for cc in range(8)]
                transpose_to(hats2, h2T, ff_tr, "trh2")

                # FFN1 (transposed): ffT[k, t] = relu(sum_c w1[c, k] h2T[c, t] + b)
                ffT = [ff_sb.tile([P, 4, TL], BF16, tag=f"ffT{i}", name=f"ffT{i}")
                       for i in range(8)]
                for kb in range(32):
                    w1t = ff_w.tile([P, C], BF16, tag="w1_ld", name=f"w1l{kb}")
                    nc.sync.dma_start(
                        out=w1t[:, :],
                        in_=w1_t[kb * P:(kb + 1) * P, :])
                    ps = ff1_psp.tile([P, TL], F32, tag="ff1_ps",
                                      name=f"ff1ps{kb}")
                    for cc in range(8):
                        nc.tensor.matmul(ps[:, :],
                                         (w1t[:, cc * P:(cc + 1) * P]),
                                         (h2T[cc][:, :]),
                                         start=(cc == 0), stop=(cc == 7))
                    nc.scalar.activation(out=ffT[kb // 4][:, kb % 4, :],
                                         in_=ps[:, :], func=AF.Relu,
                                         bias=bff1_sb[:, kb:kb + 1])

                # FFN2: out[t, c] = sum_k ffT[k, t] w2[k, c] + b2 + x2
                for n in range(2):
                    pss = [ff2_psp.tile([P, 512], F32, tag="ff2_ps",
                                        name=f"ff2ps{n}_{t}") for t in range(4)]
                    for kb in range(32):
                        w2t = ff_w.tile([P, 512], BF16, tag="w2_ld",
                                        name=f"w2l{n}_{kb}")
                        nc.sync.dma_start(
                            out=w2t[:, :],
                            in_=w2[kb * P:(kb + 1) * P, n * 512:(n + 1) * 512])
                        for t in range(4):
                            nc.tensor.matmul(
                                pss[t][:, :],
                                (ffT[kb // 4][:, kb % 4, t * P:(t + 1) * P]),
                                (w2t[:, :]),
                                start=(kb == 0), stop=(kb == 31))
                    for t in range(4):
                        yt = ff_w.tile([P, 512], F32, tag="y_ev",
                                       name=f"y{n}_{t}")
                        nc.vector.tensor_add(
                            out=yt[:, :], in0=pss[t][:, :],
                            in1=x2_tiles[t][:, n * 512:(n + 1) * 512])
                        nc.vector.tensor_add(
                            out=yt[:, :], in0=yt[:, :],
                            in1=b2_sb[:, n * 512:(n + 1) * 512])
                        nc.sync.dma_start(
                            out=out[t * P:(t + 1) * P, n * 512:(n + 1) * 512],
                            in_=yt[:, :])

        for _rep in range(reps):
            run_pipeline()

    nc.compile()
    return nc


def _block_t(w, rb, cb):
    """out[kb*128+p, cc*128+m] = w[cc*128+p, kb*128+m] (block transpose)."""
    return np.ascontiguousarray(
        w.reshape(rb, 128, cb, 128).transpose(2, 1, 0, 3).reshape(cb * 128,
                                                                  rb * 128))


def _prep_common(inputs):
    """Fold LN gains into weights; pre-transpose; cast heavies to bf16."""
    wq = np.asarray(inputs["wq"], dtype=np.float32)
    wk = np.asarray(inputs["wk"], dtype=np.float32)
    wv = np.asarray(inputs["wv"], dtype=np.float32)
    w_proj = np.asarray(inputs["w_proj"], dtype=np.float32)
    b_proj = np.asarray(inputs["b_proj"], dtype=np.float32)
    w1 = np.asarray(inputs["w1"], dtype=np.float32)
    b1 = np.asarray(inputs["b1"], dtype=np.float32)
    w2 = np.asarray(inputs["w2"], dtype=np.float32)
    b2 = np.asarray(inputs["b2"], dtype=np.float32)
    ln1_g = np.asarray(inputs["ln1_g"], dtype=np.float32)
    ln1_b = np.asarray(inputs["ln1_b"], dtype=np.float32)
    ln2_g = np.asarray(inputs["ln2_g"], dtype=np.float32)
    ln2_b = np.asarray(inputs["ln2_b"], dtype=np.float32)

    Wq = wq.transpose(1, 0, 2).reshape(C, C)   # [c, h*HS + d]
    Wk = wk.transpose(1, 0, 2).reshape(C, C)
    Wv = wv.transpose(1, 0, 2).reshape(C, C)

    s = np.arange(P)[:, None]
    q = np.arange(P)[None, :]
    common = {
        "wq_p": _block_t(ln1_g[:, None] * Wq, 8, 8).astype(BF),
        "wk_p": _block_t(ln1_g[:, None] * Wk, 8, 8).astype(BF),
        "wv_p": np.ascontiguousarray(ln1_g[:, None] * Wv).astype(BF),
        "w_proj": w_proj.astype(BF),
        "w1_t": _block_t(ln2_g[:, None] * w1, 8, 32).astype(BF),
        "w2": w2.astype(BF),
        "bias_q": np.ascontiguousarray((ln1_b @ Wq).reshape(8, P).T),
        "bias_k": np.ascontiguousarray((ln1_b @ Wk).reshape(8, P).T),
        "bias_v": (ln1_b @ Wv).reshape(1, C),
        "b_proj": b_proj.reshape(1, C),
        "bias_ff1": np.ascontiguousarray((b1 + ln2_b @ w1).reshape(32, P).T),
        "b2": b2.reshape(1, C),
        "mask_t": np.where(s <= q, 0.0, NEG).astype(np.float32),
        "ident": np.eye(P, dtype=np.float32),
    }
    return {k: np.ascontiguousarray(v) for k, v in common.items()}


def _weights_key(common):
    h = hashlib.sha1()
    for k in sorted(common):
        h.update(k.encode())
        h.update(np.ascontiguousarray(common[k]).tobytes())
    return h.hexdigest()


def _prep_inputs(inputs, common=None):
    """Per-core runtime input maps (everything that is NOT baked/gathered)."""
    x = np.asarray(inputs["x"], dtype=np.float32)
    xf = x.reshape(B * T, C)
    in_maps = []
    for i in range(N_CORES):
        m = {"x_loc": np.ascontiguousarray(xf[i * TL:(i + 1) * TL, :])}
        if WEIGHT_MODE != "const":
            if common is None:
                common = _prep_common(inputs)
            for name, rows, cols, dt in _WEIGHT_SPECS:
                rl = rows // N_CORES
                m[f"{name}_s"] = np.ascontiguousarray(
                    common[name][i * rl:(i + 1) * rl, :])
        in_maps.append(m)
    return in_maps


def kernel(**inputs):
    global _BUILT, _BUILT_KEY
    common = _prep_common(inputs)
    key = _weights_key(common) if WEIGHT_MODE == "const" else "gather"
    if _BUILT is None or _BUILT_KEY != key:
        _BUILT = _build(common)
        _BUILT_KEY = key
    in_maps = _prep_inputs(inputs, common)
    res = run_bass_kernel_spmd(_BUILT, in_maps, core_ids=list(range(N_CORES)))
    outf = np.concatenate([res.results[i]["out"] for i in range(N_CORES)],
                          axis=0)
    return outf.reshape(B, T, C).astype(np.float32)


# revision 8
# speedup vs baseline: 21.6636x; 17.3575x over previous
"""Fused transformer block (LN -> 16-head causal attention -> proj -> residual
-> LN -> FFN -> residual) on 8 TRN2 NeuronCores.

Sharding: tokens are sharded across cores for LN/QKV/proj/FFN (512 rows of the
flattened [4096, 1024] each); attention is head-sharded (2 heads per core) so
every core runs an identical SPMD program over the full causal triangle.
AllToAll collectives redistribute Q^T/K^T/V (token-sharded -> head-sharded)
and the attention output (head-sharded -> token-sharded).

All matmuls keep operands pre-transposed so the contraction dim is always the
SBUF partition dim: h and h2 are transposed on-chip via the PE; Q/K are
produced directly in [head_dim, token] layout; attention scores are computed
transposed ([key, query]) so the softmax-weighted V accumulation is a plain
PSUM matmul chain whose appended ones-column yields the softmax denominator.
LayerNorm gains are folded into the following weight matrices host-side;
LayerNorm biases become per-output-channel biases applied on PSUM eviction
or via rank-1 ones-row matmuls.

Weights are bf16 and distributed one of two ways (WEIGHT_MODE):
  "const"  — embedded in the NEFF as Const DRAM tensors, DMA'd to HBM once at
             model load. Per-execution host->device traffic is then only the
             activation input x (the weights never re-cross the slow axon
             host link).
  "gather" — staged as per-core 1/8 row-shards (each weight byte crosses the
             host link once, not 8x) and AllGathered core-to-core over
             NeuronLink into internal DRAM at kernel start.
"""

import hashlib
import os

import ml_dtypes
import numpy as np

import concourse.bass as bass  # noqa: F401  (AP helpers via handles)
import concourse.mybir as mybir
import concourse.tile as tile
from concourse import bacc

F32 = mybir.dt.float32
BF16 = mybir.dt.bfloat16
AF = mybir.ActivationFunctionType
N_CORES = 8
B, T, C = 2, 2048, 1024
H, HS = 16, 64
FF = 4 * C                # 4096
TL = (B * T) // N_CORES   # 512 local token rows per core
P = 128
LN_EPS = 1e-5
NEG = -1e9
BF = ml_dtypes.bfloat16

WEIGHT_MODE = os.environ.get("KERNEL_WEIGHT_MODE", "const")

_BUILT = None       # cache the compiled Bass module across calls
_BUILT_KEY = None   # fingerprint of the weights baked into _BUILT

# names/shapes of the weight-side tensors (everything except x), in the
# row-sharded layout used by both modes. All shard cleanly along dim 0.
_WEIGHT_SPECS = [
    # (name, rows, cols, np dtype)
    ("wq_p", C, C, BF), ("wk_p", C, C, BF), ("wv_p", C, C, BF),
    ("w_proj", C, C, BF), ("w1_t", FF, C, BF), ("w2", FF, C, BF),
]
_SMALL_SPECS = [
    ("bias_q", P, 8, np.float32), ("bias_k", P, 8, np.float32),
    ("bias_v", 1, C, np.float32), ("b_proj", 1, C, np.float32),
    ("bias_ff1", P, 32, np.float32), ("b2", 1, C, np.float32),
    ("mask_t", P, P, np.float32), ("ident", P, P, np.float32),
]


def _build(common, reps=1):
    """common: dict name -> full prepped numpy array (see _prep_common)."""
    nc = bacc.Bacc(None, target_bir_lowering=False, debug=False,
                   num_devices=N_CORES)

    # ---- external I/O (per core; bf16 to halve host-link bytes) ----
    x_loc = nc.declare_dram_parameter("x_loc", [TL, C], BF16, isOutput=False)
    out = nc.declare_dram_parameter("out", [TL, C], BF16, isOutput=True)

    dram = {}
    gathers = []  # (shard_param, gathered_tensor) pairs for gather mode
    if WEIGHT_MODE == "const":
        for name, arr in common.items():
            dram[name] = nc.inline_tensor(np.ascontiguousarray(arr),
                                          name=name)
    else:
        for name, rows, cols, dt in _WEIGHT_SPECS:
            bdt = BF16 if dt == BF else F32
            sh = nc.declare_dram_parameter(f"{name}_s", [rows // N_CORES, cols],
                                           bdt, isOutput=False)
            g = nc.dram_tensor(name, [rows, cols], bdt)
            dram[name] = g
            gathers.append((sh, g))
        for name, rows, cols, dt in _SMALL_SPECS:
            dram[name] = nc.inline_tensor(
                np.ascontiguousarray(common[name]), name=name)

    wq_p, wk_p, wv_p = dram["wq_p"], dram["wk_p"], dram["wv_p"]
    w_proj, w1_t, w2 = dram["w_proj"], dram["w1_t"], dram["w2"]
    bias_q, bias_k, bias_v = dram["bias_q"], dram["bias_k"], dram["bias_v"]
    b_proj, bias_ff1, b2 = dram["b_proj"], dram["bias_ff1"], dram["b2"]
    mask_t, ident = dram["mask_t"], dram["ident"]

    # ---- internal DRAM for collectives ----
    # QT/KT: flat [8*128, 512]; A2A slot j (rows 128j..) = head-pair j of my
    # local tokens. After A2A, rows 128r.. = my head-pair for core r's tokens.
    qk_in = nc.dram_tensor("qk_in", [8 * 2 * P, TL], BF16)
    qk_mine = nc.dram_tensor("qk_mine", [8 * 2 * P, TL], BF16)
    # V: flat [8*512, 128]; slot j = column block (head-pair j) of local V.
    v_in = nc.dram_tensor("v_in", [8 * TL, P], BF16)
    v_mine = nc.dram_tensor("v_mine", [8 * TL, P], BF16)
    # attention out: slot j = my heads' output for core j's token rows.
    attn_in = nc.dram_tensor("attn_in", [8 * TL, P], BF16)
    attn_mine = nc.dram_tensor("attn_mine", [8 * TL, P], BF16)

    groups = [list(range(N_CORES))]

    from contextlib import ExitStack
    with tile.TileContext(nc) as tc, ExitStack() as stk:
        if WEIGHT_MODE != "const":
            for sh, g in gathers:
                nc.gpsimd.collective_compute(
                    "AllGather", mybir.AluOpType.bypass,
                    replica_groups=groups, ins=[sh[:, :]], outs=[g[:, :]])

        const = stk.enter_context(tc.tile_pool(name="const", bufs=1))
        ident_sb = const.tile([P, P], F32, tag="ident", name="ident_sb")
        identb_sb = const.tile([P, P], BF16, tag="identb", name="identb_sb")
        mask_sb = const.tile([P, P], F32, tag="mask", name="mask_sb")
        eps_sb = const.tile([P, 1], F32, tag="eps", name="eps_sb")
        ones_sb = const.tile([1, P], F32, tag="ones", name="ones_sb")
        ones2_sb = const.tile([P, 2, 1], BF16, tag="ones2", name="ones2_sb")
        bq_sb = const.tile([P, 8], F32, tag="bq", name="bq_sb")
        bk_sb = const.tile([P, 8], F32, tag="bk", name="bk_sb")
        bv_sb = const.tile([P, C], F32, tag="bv", name="bv_sb")
        bproj_sb = const.tile([P, C], F32, tag="bproj", name="bproj_sb")
        b2_sb = const.tile([P, C], F32, tag="b2c", name="b2_sb")
        bff1_sb = const.tile([P, 32], F32, tag="bff1", name="bff1_sb")
        wps_c = [[const.tile([P, 512], BF16, tag=f"wpc{n}_{p}",
                             name=f"wpc{n}_{p}") for p in range(8)]
                 for n in range(2)]
        for n in range(2):
            for p in range(8):
                nc.sync.dma_start(
                    out=wps_c[n][p][:, :],
                    in_=w_proj[p * P:(p + 1) * P, n * 512:(n + 1) * 512])
        nc.sync.dma_start(out=ident_sb[:, :], in_=ident[:, :])
        nc.vector.tensor_copy(identb_sb[:, :], ident_sb[:, :])
        nc.sync.dma_start(out=mask_sb[:, :], in_=mask_t[:, :])
        nc.sync.dma_start(out=bq_sb[:, :], in_=bias_q[:, :])
        nc.sync.dma_start(out=bk_sb[:, :], in_=bias_k[:, :])
        nc.sync.dma_start(out=bv_sb[:, :], in_=bias_v[:, :].to_broadcast([P, C]))
        nc.sync.dma_start(out=bproj_sb[:, :], in_=b_proj[:, :].to_broadcast([P, C]))
        nc.sync.dma_start(out=b2_sb[:, :], in_=b2[:, :].to_broadcast([P, C]))
        nc.sync.dma_start(out=bff1_sb[:, :], in_=bias_ff1[:, :])
        nc.vector.memset(eps_sb[:, :], LN_EPS)
        nc.vector.memset(ones_sb[:, :], 1.0)
        nc.vector.memset(ones2_sb[:, :, :], 1.0)

        # x / x2 tiles stay resident for the two residual adds.
        xres = stk.enter_context(tc.tile_pool(name="xres", bufs=1))
        x_tiles = [xres.tile([P, C], F32, tag=f"x{t}", name=f"x{t}")
                   for t in range(4)]
        x2_tiles = [xres.tile([P, C], F32, tag=f"x2_{t}", name=f"x2_{t}")
                    for t in range(4)]

        def layernorm_tiles(src_tiles, dst_pool, dst_tag):
            """LN over the free dim (1024) of 4 [128, 1024] tiles."""
            hats = []
            for t in range(4):
                xt = src_tiles[t]
                stats = dst_pool.tile([P, 2, nc.vector.BN_STATS_DIM], F32,
                                      tag=f"lns{dst_tag}{t}",
                                      name=f"stats_{dst_tag}{t}")
                xg = xt[:, :].rearrange("p (s d) -> p s d", s=2)
                for s in range(2):
                    nc.vector.bn_stats(out=stats[:, s, :], in_=xg[:, s, :])
                mv = dst_pool.tile([P, nc.vector.BN_AGGR_DIM], F32,
                                   tag=f"lnm{dst_tag}{t}",
                                   name=f"mv_{dst_tag}{t}")
                nc.vector.bn_aggr(out=mv[:, :], in_=stats[:, :, :])
                nc.scalar.activation(out=mv[:, 1:2], in_=mv[:, 1:2],
                                     func=AF.Sqrt, bias=eps_sb[:, :])
                nc.vector.reciprocal(out=mv[:, 1:2], in_=mv[:, 1:2])
                # negated scaled mean for the ACT Identity pass below
                nmu = dst_pool.tile([P, 1], F32, tag=f"lnn{dst_tag}{t}",
                                    name=f"nmu_{dst_tag}{t}")
                nc.vector.tensor_tensor(out=nmu[:, :], in0=mv[:, 0:1],
                                        in1=mv[:, 1:2],
                                        op=mybir.AluOpType.mult)
                nc.vector.tensor_scalar_mul(out=nmu[:, :], in0=nmu[:, :],
                                            scalar1=-1.0)
                hat = dst_pool.tile([P, C], F32, tag=f"{dst_tag}{t}",
                                    name=f"{dst_tag}{t}")
                nc.scalar.activation(out=hat[:, :], in_=xt[:, :],
                                     func=AF.Identity, bias=nmu[:, :],
                                     scale=mv[:, 1:2])
                hats.append(hat)
            return hats

        def transpose_to(hats, dst_tiles, psum_pool, tagp):
            """4x [128, 1024] token-major -> 8x [128, 512] channel-major."""
            for cc in range(8):
                for t in range(4):
                    pt = psum_pool.tile([P, P], F32, tag=tagp,
                                        name=f"tr_{tagp}{cc}_{t}")
                    nc.tensor.transpose(pt[:, :],
                                        hats[t][:, cc * P:(cc + 1) * P],
                                        ident_sb[:, :])
                    nc.vector.tensor_copy(
                        dst_tiles[cc][:, t * P:(t + 1) * P], pt[:, :])

        def run_pipeline():
            # ================= Phase A+B: LN1, h^T, QKV =================
            with tc.tile_pool(name="pa", bufs=1) as pa, \
                 tc.tile_pool(name="pa_w", bufs=4) as pa_w, \
                 tc.tile_pool(name="pa_tr", bufs=2, space="PSUM") as pa_tr, \
                 tc.tile_pool(name="pa_mm", bufs=2, space="PSUM") as pa_mm:
                for t in range(4):
                    xb = pa_w.tile([P, C], BF16, tag="x_ld", name=f"xld{t}")
                    nc.sync.dma_start(out=xb[:, :],
                                      in_=x_loc[t * P:(t + 1) * P, :])
                    nc.vector.tensor_copy(x_tiles[t][:, :], xb[:, :])
                hats = layernorm_tiles(x_tiles, pa, "hat")
                hT = [pa.tile([P, TL], BF16, tag=f"hT{cc}", name=f"hT{cc}")
                      for cc in range(8)]
                transpose_to(hats, hT, pa_tr, "trh")

                # QT/KT[kb][:, t] = sum_c W[c, 128kb+*] hT[c, t]   (+ bias)
                for (w_dram, b_sb, off, qn) in ((wk_p, bk_sb, P, "k"),
                                                (wq_p, bq_sb, 0, "q")):
                    for kb in range(8):
                        wt = pa_w.tile([P, C], BF16, tag="wqk",
                                       name=f"wl_{qn}{kb}")
                        nc.sync.dma_start(
                            out=wt[:, :],
                            in_=w_dram[kb * P:(kb + 1) * P, :])
                        ps = pa_mm.tile([P, TL], F32, tag="qk_ps",
                                        name=f"qk_ps_{qn}{kb}")
                        for cc in range(8):
                            nc.tensor.matmul(ps[:, :],
                                             (wt[:, cc * P:(cc + 1) * P]),
                                             (hT[cc][:, :]),
                                             start=(cc == 0), stop=(cc == 7))
                        ev = pa_w.tile([P, TL], BF16, tag="qk_ev",
                                       name=f"qk_ev_{qn}{kb}")
                        nc.vector.tensor_scalar_add(out=ev[:, :], in0=ps[:, :],
                                                    scalar1=b_sb[:, kb:kb + 1])
                        nc.sync.dma_start(
                            out=qk_in[kb * 2 * P + off:kb * 2 * P + off + P, :],
                            in_=ev[:, :])

                # V[t, :] = sum_c hT[c, t] wv[c, :]  (+ bias via ones row)
                v_i3 = v_in.rearrange("(j t) d -> j t d", j=8)
                for n in range(2):
                    wvs = [pa.tile([P, 512], BF16, tag=f"wv{cc}",
                                   name=f"wv{n}_{cc}") for cc in range(8)]
                    for cc in range(8):
                        nc.sync.dma_start(
                            out=wvs[cc][:, :],
                            in_=wv_p[cc * P:(cc + 1) * P, n * 512:(n + 1) * 512])
                    for t in range(4):
                        ps = pa_mm.tile([P, 512], F32, tag="v_ps",
                                        name=f"v_ps{n}_{t}")
                        for cc in range(8):
                            nc.tensor.matmul(ps[:, :],
                                             (hT[cc][:, t * P:(t + 1) * P]),
                                             (wvs[cc][:, :]),
                                             start=(cc == 0), stop=(cc == 7))
                        ev = pa_w.tile([P, 512], BF16, tag="v_ev",
                                       name=f"v_ev{n}_{t}")
                        nc.vector.tensor_add(out=ev[:, :], in0=ps[:, :],
                                             in1=bv_sb[:, n * 512:(n + 1) * 512])
                        # scatter the 4 pair-column blocks into their A2A slots
                        for j in range(4):
                            nc.sync.dma_start(
                                out=v_i3[n * 4 + j, t * P:(t + 1) * P, :],
                                in_=ev[:, j * P:(j + 1) * P])

            # ================= Phase C: A2A QKV =================
            nc.gpsimd.collective_compute("AllToAll", mybir.AluOpType.bypass,
                                         replica_groups=groups,
                                         ins=[qk_in[:, :]], outs=[qk_mine[:, :]])
            nc.gpsimd.collective_compute("AllToAll", mybir.AluOpType.bypass,
                                         replica_groups=groups,
                                         ins=[v_in[:, :]], outs=[v_mine[:, :]])

            # ================= Phase D: attention (my 2 heads, full T) ==========
            qk_m4 = qk_mine[:, :].rearrange("(r s p) t -> r s p t", r=8, s=2)
            v_m3 = v_mine[:, :].rearrange("(r t) d -> r t d", r=8)
            attn_i3 = attn_in[:, :].rearrange("(r t) d -> r t d", r=8)

            with tc.tile_pool(name="att_kv", bufs=2) as att_kv, \
                 tc.tile_pool(name="att_sb", bufs=4) as att_sb, \
                 tc.tile_pool(name="att_sc", bufs=2, space="PSUM") as att_sc, \
                 tc.tile_pool(name="att_tr", bufs=2, space="PSUM") as att_tr, \
                 tc.tile_pool(name="att_o", bufs=2, space="PSUM") as att_o:
                for b in range(B):
                    kt_all = att_kv.tile([P, 16, P], BF16, tag="kt_all",
                                         name=f"kt_all{b}")
                    v_all = att_kv.tile([P, 16, 2, 65], BF16, tag="v_all",
                                        name=f"v_all{b}")
                    for sc in range(16):
                        g = b * 16 + sc
                        r, o = g // 4, (g % 4) * P
                        nc.sync.dma_start(out=kt_all[:, sc, :],
                                          in_=qk_m4[r, 1, :, o:o + P])
                        nc.sync.dma_start(
                            out=v_all[:, sc, :, 0:64],
                            in_=v_m3[r, o:o + P, :].rearrange(
                                "t (h d) -> t h d", h=2))
                        nc.vector.tensor_copy(v_all[:, sc, :, 64:65],
                                              ones2_sb[:, :, :])
                    for qg in range(4):
                        # q-group = 512 queries = exactly core (b*4+qg)'s tokens
                        qt_t = att_sb.tile([P, 512], BF16, tag="qt_t",
                                           name=f"qt{b}_{qg}")
                        nc.sync.dma_start(out=qt_t[:, :],
                                          in_=qk_m4[b * 4 + qg, 0, :, :])
                        att_grp = att_sb.tile([P, 4, P], BF16, tag="att_grp",
                                              name=f"ag{b}_{qg}")
                        nblocks = 4 * qg + 4
                        o_ps = [att_o.tile([65, 512], F32, tag="outT",
                                           name=f"oT{b}_{qg}_{hh}")
                                for hh in range(2)]
                        for sb in range(nblocks):
                            j = max(0, sb - 4 * qg)  # first valid q-subchunk
                            q0 = j * P
                            sc_ps = att_sc.tile([P, 2, 512], F32, tag="sc_ps",
                                                name=f"sc{b}_{qg}_{sb}")
                            for h in range(2):
                                nc.tensor.matmul(
                                    sc_ps[:, h, q0:512],
                                    (kt_all[h * 64:(h + 1) * 64, sb, :]),
                                    (qt_t[h * 64:(h + 1) * 64, q0:512]),
                                    start=True, stop=True)
                            if sb >= 4 * qg:  # diagonal sub-block needs mask
                                for h in range(2):
                                    nc.vector.tensor_add(
                                        sc_ps[:, h, q0:q0 + P],
                                        sc_ps[:, h, q0:q0 + P], mask_sb[:, :])
                            ex = att_sb.tile([P, 2, 512], BF16, tag="ex",
                                             name=f"ex{b}_{qg}_{sb}")
                            nc.scalar.activation(out=ex[:, :, q0:512],
                                                 in_=sc_ps[:, :, q0:512],
                                                 func=AF.Exp,
                                                 scale=float(C) ** -0.5)
                            for h in range(2):
                                nc.tensor.matmul(o_ps[h][:, q0:512],
                                                 (v_all[:, sb, h, :]),
                                                 (ex[:, h, q0:512]),
                                                 start=(sb == 0),
                                                 stop=(sb == nblocks - 1))
                        for h in range(2):
                            oT_sb = att_sb.tile([65, 512], F32, tag="oT_sb",
                                                name=f"oTs{b}_{qg}_{h}")
                            nc.vector.tensor_copy(oT_sb[:, :], o_ps[h][:, :])
                            for j in range(4):
                                tr_ps = att_tr.tile([P, 65], F32, tag="tr_ps",
                                                    name=f"trp{b}_{qg}_{h}_{j}")
                                nc.tensor.transpose(
                                    tr_ps[:, :], oT_sb[:, j * P:(j + 1) * P],
                                    ident_sb[0:65, 0:65])
                                rec = att_sb.tile([P, 1], F32, tag="rec",
                                                  name=f"rec{b}_{qg}_{h}_{j}")
                                nc.vector.reciprocal(rec[:, :], tr_ps[:, 64:65])
                                nc.vector.tensor_scalar_mul(
                                    out=att_grp[:, j, h * 64:(h + 1) * 64],
                                    in0=tr_ps[:, 0:64], scalar1=rec[:, :])
                        for j in range(4):
                            nc.sync.dma_start(
                                out=attn_i3[b * 4 + qg, j * P:(j + 1) * P, :],
                                in_=att_grp[:, j, :])

            # ================= Phase E: A2A attention out =================
            nc.gpsimd.collective_compute("AllToAll", mybir.AluOpType.bypass,
                                         replica_groups=groups,
                                         ins=[attn_in[:, :]],
                                         outs=[attn_mine[:, :]])
            attn_m3 = attn_mine[:, :].rearrange("(r t) d -> r t d", r=8)

            # ================= Phase F: proj + residual =================
            with tc.tile_pool(name="pr_sb", bufs=1) as pr_sb, \
                 tc.tile_pool(name="pr_w", bufs=3) as pr_w, \
                 tc.tile_pool(name="pr_tr", bufs=2, space="PSUM") as pr_tr, \
                 tc.tile_pool(name="pr_mm", bufs=2, space="PSUM") as pr_mm:
                attnT = [pr_sb.tile([P, TL], BF16, tag=f"aT{p}", name=f"aT{p}")
                         for p in range(8)]
                for p in range(8):
                    for t in range(4):
                        at = pr_w.tile([P, P], BF16, tag="at_ld",
                                       name=f"atl{p}_{t}")
                        nc.sync.dma_start(out=at[:, :],
                                          in_=attn_m3[p, t * P:(t + 1) * P, :])
                        pt = pr_tr.tile([P, P], BF16, tag="at_tr",
                                        name=f"attr{p}_{t}")
                        nc.tensor.transpose(pt[:, :], at[:, :],
                                            identb_sb[:, :])
                        nc.scalar.copy(attnT[p][:, t * P:(t + 1) * P],
                                       pt[:, :])
                for n in range(2):
                    wps = wps_c[n]
                    for t in range(4):
                        ps = pr_mm.tile([P, 512], F32, tag="pr_ps",
                                        name=f"prps{n}_{t}")
                        for p in range(8):
                            nc.tensor.matmul(ps[:, :],
                                             (attnT[p][:, t * P:(t + 1) * P]),
                                             (wps[p][:, :]),
                                             start=(p == 0), stop=(p == 7))
                        nc.vector.tensor_add(
                            out=x2_tiles[t][:, n * 512:(n + 1) * 512],
                            in0=ps[:, :],
                            in1=x_tiles[t][:, n * 512:(n + 1) * 512])
                        nc.vector.tensor_add(
                            out=x2_tiles[t][:, n * 512:(n + 1) * 512],
                            in0=x2_tiles[t][:, n * 512:(n + 1) * 512],
                            in1=bproj_sb[:, n * 512:(n + 1) * 512])

            # ================= Phase G+H+I: LN2 + FFN + residual ==============
            with tc.tile_pool(name="ff_sb", bufs=1) as ff_sb, \
                 tc.tile_pool(name="ff_w", bufs=6) as ff_w, \
                 tc.tile_pool(name="ff_tr", bufs=2, space="PSUM") as ff_tr, \
                 tc.tile_pool(name="ff1_ps", bufs=2, space="PSUM") as ff1_psp, \
                 tc.tile_pool(name="ff2_ps", bufs=4, space="PSUM") as ff2_psp:
                hats2 = layernorm_tiles(x2_tiles, ff_sb, "hat2")
                h2T = [ff_sb.tile([P, TL], BF16, tag=f"h2T{cc}", name=f"h2T{cc}")
                       for cc in range(8)]
                transpose_to(hats2, h2T, ff_tr, "trh2")

                # FFN1 (transposed): ffT[k, t] = relu(sum_c w1[c, k] h2T[c, t] + b)
                ffT = [ff_sb.tile([P, 4, TL], BF16, tag=f"ffT{i}", name=f"ffT{i}")
                       for i in range(8)]
                for kb in range(32):
                    w1t = ff_w.tile([P, C], BF16, tag="w1_ld", name=f"w1l{kb}")
                    nc.sync.dma_start(
                        out=w1t[:, :],
                        in_=w1_t[kb * P:(kb + 1) * P, :])
                    ps = ff1_psp.tile([P, TL], F32, tag="ff1_ps",
                                      name=f"ff1ps{kb}")
                    for cc in range(8):
                        nc.tensor.matmul(ps[:, :],
                                         (w1t[:, cc * P:(cc + 1) * P]),
                                         (h2T[cc][:, :]),
                                         start=(cc == 0), stop=(cc == 7))
                    nc.scalar.activation(out=ffT[kb // 4][:, kb % 4, :],
                                         in_=ps[:, :], func=AF.Relu,
                                         bias=bff1_sb[:, kb:kb + 1])

                # FFN2: out[t, c] = sum_k ffT[k, t] w2[k, c] + b2 + x2
                for n in range(2):
                    pss = [ff2_psp.tile([P, 512], F32, tag="ff2_ps",
                                        name=f"ff2ps{n}_{t}") for t in range(4)]
                    for kb in range(32):
                        w2t = ff_w.tile([P, 512], BF16, tag="w2_ld",
                                        name=f"w2l{n}_{kb}")
                        nc.sync.dma_start(
                            out=w2t[:, :],
                            in_=w2[kb * P:(kb + 1) * P, n * 512:(n + 1) * 512])
                        for t in range(4):
                            nc.tensor.matmul(
                                pss[t][:, :],
                                (ffT[kb // 4][:, kb % 4, t * P:(t + 1) * P]),
                                (w2t[:, :]),
                                start=(kb == 0), stop=(kb == 31))
                    for t in range(4):
                        yt = ff_w.tile([P, 512], F32, tag="y_ev",
                                       name=f"y{n}_{t}")
                        nc.vector.tensor_add(
                            out=yt[:, :], in0=pss[t][:, :],
                            in1=x2_tiles[t][:, n * 512:(n + 1) * 512])
                        yb = ff_w.tile([P, 512], BF16, tag="y_bf",
                                       name=f"yb{n}_{t}")
                        nc.vector.tensor_add(
                            out=yb[:, :], in0=yt[:, :],
                            in1=b2_sb[:, n * 512:(n + 1) * 512])
                        nc.sync.dma_start(
                            out=out[t * P:(t + 1) * P, n * 512:(n + 1) * 512],
                            in_=yb[:, :])

        for _rep in range(reps):
            run_pipeline()

    nc.compile()
    return nc


def _block_t(w, rb, cb):
    """out[kb*128+p, cc*128+m] = w[cc*128+p, kb*128+m] (block transpose)."""
    return np.ascontiguousarray(
        w.reshape(rb, 128, cb, 128).transpose(2, 1, 0, 3).reshape(cb * 128,
                                                                  rb * 128))


def _prep_common(inputs):
    """Fold LN gains into weights; pre-transpose; cast heavies to bf16."""
    wq = np.asarray(inputs["wq"], dtype=np.float32)
    wk = np.asarray(inputs["wk"], dtype=np.float32)
    wv = np.asarray(inputs["wv"], dtype=np.float32)
    w_proj = np.asarray(inputs["w_proj"], dtype=np.float32)
    b_proj = np.asarray(inputs["b_proj"], dtype=np.float32)
    w1 = np.asarray(inputs["w1"], dtype=np.float32)
    b1 = np.asarray(inputs["b1"], dtype=np.float32)
    w2 = np.asarray(inputs["w2"], dtype=np.float32)
    b2 = np.asarray(inputs["b2"], dtype=np.float32)
    ln1_g = np.asarray(inputs["ln1_g"], dtype=np.float32)
    ln1_b = np.asarray(inputs["ln1_b"], dtype=np.float32)
    ln2_g = np.asarray(inputs["ln2_g"], dtype=np.float32)
    ln2_b = np.asarray(inputs["ln2_b"], dtype=np.float32)

    Wq = wq.transpose(1, 0, 2).reshape(C, C)   # [c, h*HS + d]
    Wk = wk.transpose(1, 0, 2).reshape(C, C)
    Wv = wv.transpose(1, 0, 2).reshape(C, C)

    s = np.arange(P)[:, None]
    q = np.arange(P)[None, :]
    common = {
        "wq_p": _block_t(ln1_g[:, None] * Wq, 8, 8).astype(BF),
        "wk_p": _block_t(ln1_g[:, None] * Wk, 8, 8).astype(BF),
        "wv_p": np.ascontiguousarray(ln1_g[:, None] * Wv).astype(BF),
        "w_proj": w_proj.astype(BF),
        "w1_t": _block_t(ln2_g[:, None] * w1, 8, 32).astype(BF),
        "w2": w2.astype(BF),
        "bias_q": np.ascontiguousarray((ln1_b @ Wq).reshape(8, P).T),
        "bias_k": np.ascontiguousarray((ln1_b @ Wk).reshape(8, P).T),
        "bias_v": (ln1_b @ Wv).reshape(1, C),
        "b_proj": b_proj.reshape(1, C),
        "bias_ff1": np.ascontiguousarray((b1 + ln2_b @ w1).reshape(32, P).T),
        "b2": b2.reshape(1, C),
        "mask_t": np.where(s <= q, 0.0, NEG).astype(np.float32),
        "ident": np.eye(P, dtype=np.float32),
    }
    return {k: np.ascontiguousarray(v) for k, v in common.items()}


def _weights_key(common):
    h = hashlib.sha1()
    for k in sorted(common):
        h.update(k.encode())
        h.update(np.ascontiguousarray(common[k]).tobytes())
    return h.hexdigest()


def _prep_inputs(inputs, common=None):
    """Per-core runtime input maps (everything that is NOT baked/gathered)."""
    x = np.asarray(inputs["x"], dtype=np.float32).astype(BF)
    xf = x.reshape(B * T, C)
    in_maps = []
    for i in range(N_CORES):
        m = {"x_loc": np.ascontiguousarray(xf[i * TL:(i + 1) * TL, :])}
        if WEIGHT_MODE != "const":
            if common is None:
                common = _prep_common(inputs)
            for name, rows, cols, dt in _WEIGHT_SPECS:
                rl = rows // N_CORES
                m[f"{name}_s"] = np.ascontiguousarray(
                    common[name][i * rl:(i + 1) * rl, :])
        in_maps.append(m)
    return in_maps


class _Runner:
    """One-time jit of the bass_exec custom call; reused across executions.

    run_bass_via_pjrt rebuilds (and re-loads on all 8 devices) a fresh jax
    executable on EVERY call — ~5s/call for this module. Production serving
    loads the model once; this mirrors that: trace/lower/compile/load happen
    once, warm calls only move x in / out back over the host link.
    """

    def __init__(self, nc):
        import jax
        from jax.sharding import Mesh, PartitionSpec
        from jax.experimental.shard_map import shard_map
        from concourse.bass2jax import (_bass_exec_p, partition_id_tensor,
                                        install_neuronx_cc_hook)
        install_neuronx_cc_hook()

        partition_name = (nc.partition_id_tensor.name
                          if nc.partition_id_tensor else None)
        in_names, out_names, out_avals, zero_outs = [], [], [], []
        for alloc in nc.m.functions[0].allocations:
            if not isinstance(alloc, mybir.MemoryLocationSet):
                continue
            name = alloc.memorylocations[0].name
            if alloc.kind == "ExternalInput":
                if name != partition_name:
                    in_names.append(name)
            elif alloc.kind == "ExternalOutput":
                out_names.append(name)
                shape = tuple(alloc.tensor_shape)
                dtype = mybir.dt.np(alloc.dtype)
                out_avals.append(jax.core.ShapedArray(shape, dtype))
                zero_outs.append(np.zeros(shape, dtype))
        n_params = len(in_names)
        in_names_all = list(in_names) + list(out_names)
        if partition_name is not None:
            in_names_all.append(partition_name)

        def _body(*args):
            operands = list(args)
            if partition_name is not None:
                operands.append(partition_id_tensor())
            return tuple(_bass_exec_p.bind(
                *operands, out_avals=tuple(out_avals),
                in_names=tuple(in_names_all), out_names=tuple(out_names),
                lowering_input_output_aliases=(), sim_require_finite=True,
                sim_require_nnan=True, nc=nc))

        devices = jax.devices()[:N_CORES]
        mesh = Mesh(np.asarray(devices), ("core",))
        nio = n_params + len(out_names)
        self._fn = jax.jit(
            shard_map(_body, mesh=mesh,
                      in_specs=(PartitionSpec("core"),) * nio,
                      out_specs=(PartitionSpec("core"),) * len(out_names),
                      check_rep=False),
            donate_argnums=tuple(range(n_params, nio)), keep_unused=True)
        self._in_names = in_names
        self._out_names = out_names
        self._out_avals = out_avals
        self._zero_shapes = [(z.shape, z.dtype) for z in zero_outs]

    def __call__(self, in_maps):
        concat_in = [
            np.concatenate([np.asarray(m[n]) for m in in_maps], axis=0)
            for n in self._in_names]
        concat_zeros = [
            np.zeros((N_CORES * s[0], *s[1:]), d)
            for (s, d) in self._zero_shapes]
        outs = self._fn(*concat_in, *concat_zeros)
        return {
            name: np.asarray(o).reshape(N_CORES, *self._out_avals[i].shape)
            for i, (name, o) in enumerate(zip(self._out_names, outs))}


_RUNNER = None


def kernel(**inputs):
    global _BUILT, _BUILT_KEY, _RUNNER
    common = _prep_common(inputs)
    key = _weights_key(common) if WEIGHT_MODE == "const" else "gather"
    if _BUILT is None or _BUILT_KEY != key:
        _BUILT = _build(common)
        _BUILT_KEY = key
        _RUNNER = _Runner(_BUILT)
    in_maps = _prep_inputs(inputs, common)
    res = _RUNNER(in_maps)
    outf = res["out"].reshape(B * T, C)
    return outf.reshape(B, T, C).astype(np.float32)
